# revision 1
# baseline (speedup 1.0000x reference)
"""KANLinear (B-spline) Trainium2 kernel.

Math: out = silu(x) @ Wb^T + einsum('nik,oik->no', Bspline(x), Ws*scaler)
Grid is uniform: knots at t = j (j=0..11) where t = x/1.2 + 5.5.
Closed form per element: m = floor(t) in [0,10], u = t - m,
  b_k = mask[m==k]*P0(u) + mask[m==k+1]*P1(u) + mask[m==k+2]*P2(u) + mask[m==k+3]*P3(u)
  P0 = u^3/6, P1 = (-3u^3+3u^2+3u+1)/6, P2 = (3u^3-6u^2+4)/6, P3 = (1-u)^3/6
Sharding: data-parallel over N across 8 cores; weights replicated.
GEMM: K = 512 (silu base) + 512*8 (spline) = 4608, bf16 inputs, f32 PSUM.
"""
import sys, os
sys.path.insert(0, '/opt/trn_rl_repo')
import numpy as np
import ml_dtypes
from contextlib import ExitStack

import concourse.bass as bass
import concourse.bacc as bacc
import concourse.tile as tile
import concourse.mybir as mybir
from concourse.bass_utils import run_bass_kernel_spmd

f32 = mybir.dt.float32
bf16 = mybir.dt.bfloat16
Alu = mybir.AluOpType
Act = mybir.ActivationFunctionType

N_TOTAL, IN_F, OUT_F = 32768, 512, 512
NCORES = 8
N_CORE = N_TOTAL // NCORES          # 4096
NBLK = 512                          # rows per block
NBLOCKS = N_CORE // NBLK            # 8
KT = 4 + 8 * 4                      # 36 K-tiles of 128: 4 silu + 8 coef * 4 i-tiles
INV_H = 1.0 / 1.2
T_OFF = 5.5

_cache = {}


def _build():
    if 'nc' in _cache:
        return _cache['nc']
    nc = bacc.Bacc("TRN2", target_bir_lowering=False, debug=False, num_devices=NCORES)
    for cv in (T_OFF, INV_H, -6.6, 1.2):
        th = nc.alloc_sbuf_tensor(f"constk-{cv}", [128, 1], f32)
        nc.gpsimd.memset(th.ap(), cv)
        nc.const_aps.aps[(f32, cv)] = th.ap()
    nc.all_engine_barrier()
    x_d = nc.dram_tensor("x", [N_CORE, IN_F], f32, kind="ExternalInput").ap()
    w_d = nc.dram_tensor("w", [KT * 128, OUT_F], bf16, kind="ExternalInput").ap()
    id_d = nc.dram_tensor("ident", [128, 128], f32, kind="ExternalInput").ap()
    y_d = nc.dram_tensor("y", [N_CORE, OUT_F], f32, kind="ExternalOutput").ap()

    with tile.TileContext(nc) as tc, ExitStack() as ctx:
        wpool = ctx.enter_context(tc.tile_pool(name="w", bufs=1))
        xpool = ctx.enter_context(tc.tile_pool(name="x", bufs=3))
        npool = ctx.enter_context(tc.tile_pool(name="tnat", bufs=8))
        tpool = ctx.enter_context(tc.tile_pool(name="tT", bufs=2))
        kpool = ctx.enter_context(tc.tile_pool(name="ktiles", bufs=1))
        tmp = ctx.enter_context(tc.tile_pool(name="tmp", bufs=1))
        pt_pool = ctx.enter_context(tc.tile_pool(name="ptrans", bufs=2, space="PSUM"))
        po_pool = ctx.enter_context(tc.tile_pool(name="pout", bufs=4, space="PSUM"))

        w_s = wpool.tile([128, KT * OUT_F], bf16, tag="w")
        ident = wpool.tile([128, 128], f32, tag="ident")
        nc.sync.dma_start(ident[:], id_d[:])
        for kt in range(KT):
            nc.sync.dma_start(w_s[:, kt * OUT_F:(kt + 1) * OUT_F],
                              w_d[kt * 128:(kt + 1) * 128, :])

        for blk in range(NBLOCKS):
            r0 = blk * NBLK
            # load x block and compute t = relu(x/1.2 + 5.5) in natural layout
            tnat = []
            for nt in range(4):
                xt = xpool.tile([128, IN_F], f32, tag="xin")
                nc.sync.dma_start(xt[:], x_d[r0 + nt * 128: r0 + (nt + 1) * 128, :])
                tn = npool.tile([128, IN_F], f32, tag="tnat")
                nc.scalar.activation(tn[:], xt[:], Act.Relu, bias=T_OFF, scale=INV_H)
                tnat.append(tn)
            # K-tile buffer for this block: [128, KT*NBLK] bf16
            kbuf = kpool.tile([128, KT * NBLK], bf16, tag="kbuf")

            for it in range(4):
                # transpose t[:, it*128:+128] from all 4 n-tiles -> tT [128i, 512n]
                ptr = pt_pool.tile([128, NBLK], f32, tag="ptr")
                for nt in range(4):
                    nc.tensor.transpose(ptr[:, nt * 128:(nt + 1) * 128],
                                        tnat[nt][:, it * 128:(it + 1) * 128], ident[:])
                tT = tpool.tile([128, NBLK], f32, tag="tT")
                nc.scalar.copy(tT[:], ptr[:])

                # silu K-tile: silu(x) = silu(1.2*t - 6.6)
                nc.scalar.activation(kbuf[:, it * NBLK:(it + 1) * NBLK], tT[:],
                                     Act.Silu, bias=-6.6, scale=1.2)

                # clamped t, u, m
                tcl = tmp.tile([128, NBLK], f32, tag="tcl")
                nc.vector.tensor_scalar(tcl[:], tT[:], 10.9999, None, Alu.min)
                # steps g_j = [t >= j], m = sum(g_j), u = t - m  (no mod/floor in ISA)
                g = []
                for j in range(1, 11):
                    gj = tmp.tile([128, NBLK], f32, tag=f"g{j}")
                    nc.vector.tensor_scalar(gj[:], tcl[:], float(j), None, Alu.is_ge)
                    g.append(gj)
                macc = tmp.tile([128, NBLK], f32, tag="macc0")
                nc.vector.tensor_add(macc[:], g[0][:], g[1][:])
                for j in range(2, 10):
                    nmacc = tmp.tile([128, NBLK], f32, tag=f"macc{(j-1) % 2}")
                    nc.vector.tensor_add(nmacc[:], macc[:], g[j][:])
                    macc = nmacc
                u = tmp.tile([128, NBLK], f32, tag="u")
                nc.vector.tensor_sub(u[:], tcl[:], macc[:])
                u2 = tmp.tile([128, NBLK], f32, tag="u2")
                nc.vector.tensor_mul(u2[:], u[:], u[:])
                u3 = tmp.tile([128, NBLK], f32, tag="u3")
                nc.vector.tensor_mul(u3[:], u2[:], u[:])

                # cubic pieces
                P0 = tmp.tile([128, NBLK], f32, tag="P0")
                nc.vector.tensor_scalar(P0[:], u3[:], 1.0 / 6.0, None, Alu.mult)
                s_ = tmp.tile([128, NBLK], f32, tag="s_")
                nc.vector.tensor_add(s_[:], u[:], u2[:])
                q1 = tmp.tile([128, NBLK], f32, tag="q1")
                nc.vector.tensor_scalar(q1[:], s_[:], 0.5, 1.0 / 6.0, Alu.mult, Alu.add)
                P1 = tmp.tile([128, NBLK], f32, tag="P1")
                nc.vector.scalar_tensor_tensor(P1[:], u3[:], -0.5, q1[:], Alu.mult, Alu.add)
                q2 = tmp.tile([128, NBLK], f32, tag="q2")
                nc.vector.tensor_scalar(q2[:], u2[:], -1.0, 2.0 / 3.0, Alu.mult, Alu.add)
                P2 = tmp.tile([128, NBLK], f32, tag="P2")
                nc.vector.scalar_tensor_tensor(P2[:], u3[:], 0.5, q2[:], Alu.mult, Alu.add)
                dq = tmp.tile([128, NBLK], f32, tag="dq")
                nc.vector.tensor_sub(dq[:], u2[:], u[:])
                q3 = tmp.tile([128, NBLK], f32, tag="q3")
                nc.vector.tensor_scalar(q3[:], dq[:], 0.5, 1.0 / 6.0, Alu.mult, Alu.add)
                P3 = tmp.tile([128, NBLK], f32, tag="P3")
                nc.vector.scalar_tensor_tensor(P3[:], u3[:], -1.0 / 6.0, q3[:], Alu.mult, Alu.add)

                # interval masks: mask_j = g_j - g_{j+1}; ends from step complements
                masks = []
                m0 = tmp.tile([128, NBLK], f32, tag="m0")
                nc.gpsimd.tensor_scalar(m0[:], g[0][:], -1.0, 1.0, Alu.mult, Alu.add)
                masks.append(m0)
                for j in range(1, 10):
                    mj = tmp.tile([128, NBLK], f32, tag=f"m{j}")
                    nc.gpsimd.tensor_sub(mj[:], g[j - 1][:], g[j][:])
                    masks.append(mj)
                masks.append(g[9])

                # combine: b_k -> kbuf tile (4 + k*4 + it)
                for k in range(8):
                    t1 = tmp.tile([128, NBLK], f32, tag="t1")
                    nc.vector.tensor_mul(t1[:], masks[k][:], P0[:])
                    t2 = tmp.tile([128, NBLK], f32, tag="t2")
                    nc.vector.tensor_mul(t2[:], masks[k + 1][:], P1[:])
                    t12 = tmp.tile([128, NBLK], f32, tag="t12")
                    nc.vector.tensor_add(t12[:], t1[:], t2[:])
                    t3 = tmp.tile([128, NBLK], f32, tag="t3")
                    nc.gpsimd.tensor_mul(t3[:], masks[k + 2][:], P2[:])
                    t4 = tmp.tile([128, NBLK], f32, tag="t4")
                    nc.gpsimd.tensor_mul(t4[:], masks[k + 3][:], P3[:])
                    t34 = tmp.tile([128, NBLK], f32, tag="t34")
                    nc.vector.tensor_add(t34[:], t3[:], t4[:])
                    kslot = 4 + k * 4 + it
                    nc.vector.tensor_add(kbuf[:, kslot * NBLK:(kslot + 1) * NBLK],
                                         t12[:], t34[:])

            # GEMM: for each n-sub row tile accumulate over all K tiles
            for nsub in range(4):
                po = po_pool.tile([128, OUT_F], f32, tag="po")
                for kt in range(KT):
                    nc.tensor.matmul(
                        po[:],
                        kbuf[:, kt * NBLK + nsub * 128: kt * NBLK + (nsub + 1) * 128],
                        w_s[:, kt * OUT_F:(kt + 1) * OUT_F],
                        start=(kt == 0), stop=(kt == KT - 1))
                yo = xpool.tile([128, OUT_F], f32, tag="yout")
                nc.scalar.copy(yo[:], po[:])
                nc.sync.dma_start(y_d[r0 + nsub * 128: r0 + (nsub + 1) * 128, :], yo[:])

    nc.compile()
    _cache['nc'] = nc
    return nc


def _prep_w(base_weight, spline_weight, spline_scaler):
    sw = spline_weight * spline_scaler[..., None]        # [out, in, 8]
    w = np.zeros((KT * 128, OUT_F), dtype=np.float32)
    w[0:512, :] = base_weight.T                          # silu branch
    for k in range(8):
        for it in range(4):
            kslot = 4 + k * 4 + it
            w[kslot * 128:(kslot + 1) * 128, :] = sw[:, it * 128:(it + 1) * 128, k].T
    return w.astype(ml_dtypes.bfloat16)


def kernel(x, base_weight, spline_weight, spline_scaler, grid):
    x = np.asarray(x, dtype=np.float32)
    w = _prep_w(np.asarray(base_weight, np.float32),
                np.asarray(spline_weight, np.float32),
                np.asarray(spline_scaler, np.float32))
    ident = np.eye(128, dtype=np.float32)
    nc = _build()
    in_maps = []
    for c in range(NCORES):
        in_maps.append({"x": np.ascontiguousarray(x[c * N_CORE:(c + 1) * N_CORE]),
                        "w": w, "ident": ident})
    res = run_bass_kernel_spmd(nc, in_maps, core_ids=list(range(NCORES)))
    out = np.concatenate([res.results[c]["y"] for c in range(NCORES)], axis=0)
    return out.astype(np.float32)



# revision 2
# speedup vs baseline: 81.5561x; 81.5561x over previous
"""KANLinear (B-spline) Trainium2 kernel.

Math: out = silu(x) @ Wb^T + einsum('nik,oik->no', Bspline(x), Ws*scaler)
Grid is uniform: knots at t = j (j=0..11) where t = x/1.2 + 5.5.
Closed form per element: m = floor(t) in [0,10], u = t - m,
  b_k = mask[m==k]*P0(u) + mask[m==k+1]*P1(u) + mask[m==k+2]*P2(u) + mask[m==k+3]*P3(u)
  P0 = u^3/6, P1 = (-3u^3+3u^2+3u+1)/6, P2 = (3u^3-6u^2+4)/6, P3 = (1-u)^3/6
Sharding: data-parallel over N across 8 cores; weights replicated.
GEMM: K = 512 (silu base) + 512*8 (spline) = 4608, bf16 inputs, f32 PSUM.
"""
import sys, os
sys.path.insert(0, '/opt/trn_rl_repo')
import numpy as np
import ml_dtypes
from contextlib import ExitStack

import concourse.bass as bass
import concourse.bacc as bacc
import concourse.tile as tile
import concourse.mybir as mybir
from concourse.bass_utils import run_bass_kernel_spmd

f32 = mybir.dt.float32
bf16 = mybir.dt.bfloat16
Alu = mybir.AluOpType
Act = mybir.ActivationFunctionType

N_TOTAL, IN_F, OUT_F = 32768, 512, 512
NCORES = 8
N_CORE = N_TOTAL // NCORES          # 4096
NBLK = 512                          # rows per block
NBLOCKS = N_CORE // NBLK            # 8
KT = 4 + 8 * 4                      # 36 K-tiles of 128: 4 silu + 8 coef * 4 i-tiles
INV_H = 1.0 / 1.2
T_OFF = 5.5

_cache = {}


def _build():
    if 'nc' in _cache:
        return _cache['nc']
    nc = bacc.Bacc("TRN2", target_bir_lowering=False, debug=False, num_devices=NCORES)
    for cv in (T_OFF, INV_H, -6.6, 1.2):
        th = nc.alloc_sbuf_tensor(f"constk-{cv}", [128, 1], f32)
        nc.gpsimd.memset(th.ap(), cv)
        nc.const_aps.aps[(f32, cv)] = th.ap()
    nc.all_engine_barrier()
    x_d = nc.dram_tensor("x", [N_CORE, IN_F], f32, kind="ExternalInput").ap()
    w_d = nc.dram_tensor("w", [KT * 128, OUT_F], bf16, kind="ExternalInput").ap()
    id_d = nc.dram_tensor("ident", [128, 128], f32, kind="ExternalInput").ap()
    y_d = nc.dram_tensor("y", [N_CORE, OUT_F], f32, kind="ExternalOutput").ap()

    with tile.TileContext(nc) as tc, ExitStack() as ctx:
        wpool = ctx.enter_context(tc.tile_pool(name="w", bufs=1))
        xpool = ctx.enter_context(tc.tile_pool(name="x", bufs=3))
        npool = ctx.enter_context(tc.tile_pool(name="tnat", bufs=8))
        tpool = ctx.enter_context(tc.tile_pool(name="tT", bufs=2))
        kpool = ctx.enter_context(tc.tile_pool(name="ktiles", bufs=1))
        tmp = ctx.enter_context(tc.tile_pool(name="tmp", bufs=1))
        pt_pool = ctx.enter_context(tc.tile_pool(name="ptrans", bufs=2, space="PSUM"))
        po_pool = ctx.enter_context(tc.tile_pool(name="pout", bufs=4, space="PSUM"))

        w_s = wpool.tile([128, KT * OUT_F], bf16, tag="w")
        ident = wpool.tile([128, 128], f32, tag="ident")
        nc.sync.dma_start(ident[:], id_d[:])
        for kt in range(KT):
            nc.sync.dma_start(w_s[:, kt * OUT_F:(kt + 1) * OUT_F],
                              w_d[kt * 128:(kt + 1) * 128, :])

        for blk in range(NBLOCKS):
            r0 = blk * NBLK
            # load x block and compute t = relu(x/1.2 + 5.5) in natural layout
            tnat = []
            for nt in range(4):
                xt = xpool.tile([128, IN_F], f32, tag="xin")
                nc.sync.dma_start(xt[:], x_d[r0 + nt * 128: r0 + (nt + 1) * 128, :])
                tn = npool.tile([128, IN_F], f32, tag="tnat")
                nc.scalar.activation(tn[:], xt[:], Act.Relu, bias=T_OFF, scale=INV_H)
                tnat.append(tn)
            # K-tile buffer for this block: [128, KT*NBLK] bf16
            kbuf = kpool.tile([128, KT * NBLK], bf16, tag="kbuf")

            for it in range(4):
                # transpose t[:, it*128:+128] from all 4 n-tiles -> tT [128i, 512n]
                ptr = pt_pool.tile([128, NBLK], f32, tag="ptr")
                for nt in range(4):
                    nc.tensor.transpose(ptr[:, nt * 128:(nt + 1) * 128],
                                        tnat[nt][:, it * 128:(it + 1) * 128], ident[:])
                tT = tpool.tile([128, NBLK], f32, tag="tT")
                nc.scalar.copy(tT[:], ptr[:])

                # silu K-tile: silu(x) = silu(1.2*t - 6.6)
                nc.scalar.activation(kbuf[:, it * NBLK:(it + 1) * NBLK], tT[:],
                                     Act.Silu, bias=-6.6, scale=1.2)

                # clamped t, u, m
                tcl = tmp.tile([128, NBLK], f32, tag="tcl")
                nc.vector.tensor_scalar(tcl[:], tT[:], 10.9999, None, Alu.min)
                # steps g_j = [t >= j], m = sum(g_j), u = t - m  (no mod/floor in ISA)
                g = []
                for j in range(1, 11):
                    gj = tmp.tile([128, NBLK], f32, tag=f"g{j}")
                    nc.vector.tensor_scalar(gj[:], tcl[:], float(j), None, Alu.is_ge)
                    g.append(gj)
                macc = tmp.tile([128, NBLK], f32, tag="macc0")
                nc.vector.tensor_add(macc[:], g[0][:], g[1][:])
                for j in range(2, 10):
                    nmacc = tmp.tile([128, NBLK], f32, tag=f"macc{(j-1) % 2}")
                    nc.vector.tensor_add(nmacc[:], macc[:], g[j][:])
                    macc = nmacc
                u = tmp.tile([128, NBLK], f32, tag="u")
                nc.vector.tensor_sub(u[:], tcl[:], macc[:])
                u2 = tmp.tile([128, NBLK], f32, tag="u2")
                nc.vector.tensor_mul(u2[:], u[:], u[:])
                u3 = tmp.tile([128, NBLK], f32, tag="u3")
                nc.vector.tensor_mul(u3[:], u2[:], u[:])

                # cubic pieces
                P0 = tmp.tile([128, NBLK], f32, tag="P0")
                nc.vector.tensor_scalar(P0[:], u3[:], 1.0 / 6.0, None, Alu.mult)
                s_ = tmp.tile([128, NBLK], f32, tag="s_")
                nc.vector.tensor_add(s_[:], u[:], u2[:])
                q1 = tmp.tile([128, NBLK], f32, tag="q1")
                nc.vector.tensor_scalar(q1[:], s_[:], 0.5, 1.0 / 6.0, Alu.mult, Alu.add)
                P1 = tmp.tile([128, NBLK], f32, tag="P1")
                nc.vector.scalar_tensor_tensor(P1[:], u3[:], -0.5, q1[:], Alu.mult, Alu.add)
                q2 = tmp.tile([128, NBLK], f32, tag="q2")
                nc.vector.tensor_scalar(q2[:], u2[:], -1.0, 2.0 / 3.0, Alu.mult, Alu.add)
                P2 = tmp.tile([128, NBLK], f32, tag="P2")
                nc.vector.scalar_tensor_tensor(P2[:], u3[:], 0.5, q2[:], Alu.mult, Alu.add)
                dq = tmp.tile([128, NBLK], f32, tag="dq")
                nc.vector.tensor_sub(dq[:], u2[:], u[:])
                q3 = tmp.tile([128, NBLK], f32, tag="q3")
                nc.vector.tensor_scalar(q3[:], dq[:], 0.5, 1.0 / 6.0, Alu.mult, Alu.add)
                P3 = tmp.tile([128, NBLK], f32, tag="P3")
                nc.vector.scalar_tensor_tensor(P3[:], u3[:], -1.0 / 6.0, q3[:], Alu.mult, Alu.add)

                # interval masks: mask_j = g_j - g_{j+1}; ends from step complements
                masks = []
                m0 = tmp.tile([128, NBLK], f32, tag="m0")
                nc.gpsimd.tensor_scalar(m0[:], g[0][:], -1.0, 1.0, Alu.mult, Alu.add)
                masks.append(m0)
                for j in range(1, 10):
                    mj = tmp.tile([128, NBLK], f32, tag=f"m{j}")
                    nc.gpsimd.tensor_sub(mj[:], g[j - 1][:], g[j][:])
                    masks.append(mj)
                masks.append(g[9])

                # combine: b_k -> kbuf tile (4 + k*4 + it)
                for k in range(8):
                    t1 = tmp.tile([128, NBLK], f32, tag="t1")
                    nc.vector.tensor_mul(t1[:], masks[k][:], P0[:])
                    t2 = tmp.tile([128, NBLK], f32, tag="t2")
                    nc.vector.tensor_mul(t2[:], masks[k + 1][:], P1[:])
                    t12 = tmp.tile([128, NBLK], f32, tag="t12")
                    nc.vector.tensor_add(t12[:], t1[:], t2[:])
                    t3 = tmp.tile([128, NBLK], f32, tag="t3")
                    nc.gpsimd.tensor_mul(t3[:], masks[k + 2][:], P2[:])
                    t4 = tmp.tile([128, NBLK], f32, tag="t4")
                    nc.gpsimd.tensor_mul(t4[:], masks[k + 3][:], P3[:])
                    t34 = tmp.tile([128, NBLK], f32, tag="t34")
                    nc.vector.tensor_add(t34[:], t3[:], t4[:])
                    kslot = 4 + k * 4 + it
                    nc.vector.tensor_add(kbuf[:, kslot * NBLK:(kslot + 1) * NBLK],
                                         t12[:], t34[:])

            # GEMM: for each n-sub row tile accumulate over all K tiles
            for nsub in range(4):
                po = po_pool.tile([128, OUT_F], f32, tag="po")
                for kt in range(KT):
                    nc.tensor.matmul(
                        po[:],
                        kbuf[:, kt * NBLK + nsub * 128: kt * NBLK + (nsub + 1) * 128],
                        w_s[:, kt * OUT_F:(kt + 1) * OUT_F],
                        start=(kt == 0), stop=(kt == KT - 1))
                yo = xpool.tile([128, OUT_F], f32, tag="yout")
                nc.scalar.copy(yo[:], po[:])
                nc.sync.dma_start(y_d[r0 + nsub * 128: r0 + (nsub + 1) * 128, :], yo[:])

    nc.compile()
    _cache['nc'] = nc
    return nc


def _prep_w(base_weight, spline_weight, spline_scaler):
    sw = spline_weight * spline_scaler[..., None]        # [out, in, 8]
    w = np.zeros((KT * 128, OUT_F), dtype=np.float32)
    w[0:512, :] = base_weight.T                          # silu branch
    for k in range(8):
        for it in range(4):
            kslot = 4 + k * 4 + it
            w[kslot * 128:(kslot + 1) * 128, :] = sw[:, it * 128:(it + 1) * 128, k].T
    return w.astype(ml_dtypes.bfloat16)


def _make_in_maps(inputs):
    x = np.asarray(inputs["x"], dtype=np.float32)
    w = _prep_w(np.asarray(inputs["base_weight"], np.float32),
                np.asarray(inputs["spline_weight"], np.float32),
                np.asarray(inputs["spline_scaler"], np.float32))
    ident = np.eye(128, dtype=np.float32)
    in_maps = []
    for c in range(NCORES):
        in_maps.append({"x": np.ascontiguousarray(x[c * N_CORE:(c + 1) * N_CORE]),
                        "w": w, "ident": ident})
    return in_maps


def _gather(res):
    out = np.concatenate([res.results[c]["y"] for c in range(NCORES)], axis=0)
    return out.astype(np.float32)


def kernel(x, base_weight, spline_weight, spline_scaler, grid):
    nc = _build()
    in_maps = _make_in_maps({"x": x, "base_weight": base_weight,
                             "spline_weight": spline_weight,
                             "spline_scaler": spline_scaler})
    res = run_bass_kernel_spmd(nc, in_maps, core_ids=list(range(NCORES)))
    return _gather(res)

